# revision 27
# baseline (speedup 1.0000x reference)
"""Block-ELL sparse linear layer on 8 Trainium2 NeuronCores.

Strategy: data-parallel over tokens (1024 tokens/core). The host scatters
the block-sparse values into a dense [4096, 4096] weight matrix (25%
nonzero), pre-transposes x, and each core runs a dense
  yT[out, tok] = sum_f  W[f, out]^T-as-lhsT  @  xT[f, tok]
tiled matmul with PSUM accumulation over the 32 contraction slices.
"""

import numpy as np
import ml_dtypes
from contextlib import ExitStack

import concourse.bass as bass
import concourse.bacc as bacc
import concourse.tile as tile
from concourse import mybir
from concourse.bass_utils import run_bass_kernel_spmd

# Problem constants (hardcoded per spec)
N_TOK = 8192
R = 256  # out block-rows
C = 256  # in block-cols
K = 64   # kept blocks per row
B = 16   # block size
D_IN = C * B   # 4096
D_OUT = R * B  # 4096
NCORES = 8
TOK = N_TOK // NCORES  # 1024 tokens per core
TT = 2                 # token tiles per core (512 each)
TOKT = TOK // TT       # 512

# matmul dtype: "bf16" or "f32r"
MM_DTYPE = "bf16"

_cache = {}


def _build_program(feat_tiles: int, dt_name: str):
    """Build the SPMD single-core program. Returns (nc, names)."""
    key = (feat_tiles, dt_name)
    if key in _cache:
        return _cache[key]

    if dt_name == "bf16":
        mmdt = mybir.dt.bfloat16
    else:
        mmdt = mybir.dt.float32r
    f32 = mybir.dt.float32

    FEAT = feat_tiles          # contraction tiles of 128
    OGB = 16                   # out-group blocks (256 outs each)
    OG2 = 2                    # 128-out groups per block

    nc = bacc.Bacc("TRN2", target_bir_lowering=False, debug=False,
                   num_devices=NCORES)

    xT_d = nc.dram_tensor("xT", [FEAT * 128, TOK], mmdt, kind="ExternalInput").ap()
    # W pre-tiled on host: [OGB//2, FEAT, 128, 512] — one tile serves two
    # consecutive out-blocks (halves DMA and PE-wait counts)
    w_d = nc.dram_tensor("W", [OGB // 2, FEAT, 128, 512], mmdt,
                         kind="ExternalInput").ap()
    # og-major output so every store is contiguous; host reassembles
    yT_d = nc.dram_tensor("yT", [OGB * OG2, 128, TOK], f32,
                          kind="ExternalOutput").ap()

    with tile.TileContext(nc) as tc, ExitStack() as ctx:
        xpool = ctx.enter_context(tc.tile_pool(name="x", bufs=1))
        wpool = ctx.enter_context(tc.tile_pool(name="w", bufs=36))
        ppool = ctx.enter_context(tc.tile_pool(name="ps", bufs=2, space="PSUM"))
        ypool = ctx.enter_context(tc.tile_pool(name="y", bufs=3))

        # resident x^T: [128, FEAT*TOK], slice f at cols f*TOK..(f+1)*TOK
        xt = xpool.tile([128, FEAT * TOK], mmdt)
        xT_r = xT_d.rearrange("(f p) n -> f p n", p=128)
        for f in range(FEAT):
            nc.sync.dma_start(out=xt[:, f * TOK:(f + 1) * TOK], in_=xT_r[f])

        # PE warmup while x streams in: absorbs the cold HAM clock-gate
        # phase (~3.4us at half rate) with throwaway work so the first real
        # matmuls run at full speed. Longer warmups serialize ahead of real
        # work (PE FIFO) and regress.
        NWARM = 72
        wj = xpool.tile([128, 128], mmdt, name="wj")
        nc.vector.memset(wj[:], 0.0)
        wu = ppool.tile([128, 128], f32, name="wu", tag="ps0")
        for i in range(NWARM):
            nc.tensor.matmul(wu[:], wj[:], wj[:, :128],
                             start=(i == 0), stop=(i == NWARM - 1))

        wts = {}
        for ogb in range(OGB):
            ps = [ppool.tile([128, TOKT], f32, name=f"ps_{ogb}_{i}", tag=f"ps{i}")
                  for i in range(4)]
            for f in range(FEAT):
                if ogb % 2 == 0:
                    # W streams on the scalar HWDGE ring so it is not queued
                    # behind the resident-x loads on the sync ring
                    wt = wpool.tile([128, 512], mmdt,
                                    name=f"wt_{ogb // 2}_{f}", tag="wt")
                    wts[f] = wt
                    nc.scalar.dma_start(out=wt[:], in_=w_d[ogb // 2, f])
                else:
                    wt = wts[f]
                half = (ogb % 2) * 256
                for og2 in range(OG2):
                    for t in range(TT):
                        nc.tensor.matmul(
                            ps[og2 * TT + t][:],
                            wt[:, half + og2 * 128:half + (og2 + 1) * 128],
                            xt[:, f * TOK + t * TOKT: f * TOK + (t + 1) * TOKT],
                            start=(f == 0), stop=(f == FEAT - 1),
                        )
            for og2 in range(OG2):
                yt = ypool.tile([128, TOK], f32, name=f"yt_{ogb}_{og2}", tag="yt")
                og = ogb * OG2 + og2
                for t in range(TT):
                    if (og2 * TT + t) % 2 == 0:
                        nc.vector.tensor_copy(yt[:, t * TOKT:(t + 1) * TOKT],
                                              ps[og2 * TT + t][:])
                    else:
                        nc.scalar.copy(yt[:, t * TOKT:(t + 1) * TOKT],
                                       ps[og2 * TT + t][:])
                    # per-half store so the final DMA starts as soon as its
                    # eviction lands rather than after both halves
                    nc.sync.dma_start(
                        out=yT_d[og, :, t * TOKT:(t + 1) * TOKT],
                        in_=yt[:, t * TOKT:(t + 1) * TOKT])

    nc.compile()
    _cache[key] = nc
    return nc


def _scatter_dense(values: np.ndarray, col_indices: np.ndarray) -> np.ndarray:
    """W[c*16+i, r*16+o] = sum_{k: col[r,k]=c} values[r,k,o,i]."""
    Wd = np.zeros((C, B, R, B), np.float32)  # [c, i, r, o]
    vT = np.ascontiguousarray(values.transpose(0, 1, 3, 2))  # [r, k, i, o]
    for r in range(R):
        np.add.at(Wd[:, :, r, :], (col_indices[r],), vT[r])
    return Wd.reshape(D_IN, D_OUT)


def _run(x, values, bias, col_indices, trace=False):
    x = np.asarray(x, np.float32)
    values = np.asarray(values, np.float32)
    bias = np.asarray(bias, np.float32)
    col_indices = np.asarray(col_indices, np.int32)

    W = _scatter_dense(values, col_indices)  # [D_IN, D_OUT] fp32
    has_bias = bool(np.any(bias))
    FEAT = D_IN // 128 + (1 if has_bias else 0)

    # augment contraction with a bias row if needed
    xT = np.ascontiguousarray(x.T)  # [D_IN, N_TOK]
    if has_bias:
        xT = np.concatenate([xT, np.zeros((128, N_TOK), np.float32)], 0)
        xT[D_IN, :] = 1.0
        W = np.concatenate([W, np.zeros((128, D_OUT), np.float32)], 0)
        W[D_IN, :] = bias

    np_dt = ml_dtypes.bfloat16 if MM_DTYPE == "bf16" else np.float32
    # pre-tile W: [OGB//2, FEAT, 128, 512]
    Wt = np.ascontiguousarray(
        W.reshape(FEAT, 128, 8, 512).transpose(2, 0, 1, 3)).astype(np_dt)
    xTc = xT.astype(np_dt)

    nc = _build_program(FEAT, MM_DTYPE)

    in_maps = []
    for c in range(NCORES):
        shard = np.ascontiguousarray(xTc[:, c * TOK:(c + 1) * TOK])
        in_maps.append({"xT": shard, "W": Wt})

    res = run_bass_kernel_spmd(nc, in_maps, list(range(NCORES)), trace=trace)

    y = np.empty((N_TOK, D_OUT), np.float32)
    for c in range(NCORES):
        yT = res.results[c]["yT"].reshape(D_OUT, TOK)
        y[c * TOK:(c + 1) * TOK, :] = yT.T
    return y, res


def kernel(x: np.ndarray, values: np.ndarray, bias: np.ndarray,
           col_indices: np.ndarray) -> np.ndarray:
    return _run(x, values, bias, col_indices)[0]


def run_traced(x, values, bias, col_indices):
    return _run(x, values, bias, col_indices, trace=True)[1]


# revision 28
# speedup vs baseline: 1.0173x; 1.0173x over previous
"""Block-ELL sparse linear layer on 8 Trainium2 NeuronCores.

Strategy: data-parallel over tokens (1024 tokens/core). The host scatters
the block-sparse values into a dense [4096, 4096] weight matrix (25%
nonzero), pre-transposes x, and each core runs a dense
  yT[out, tok] = sum_f  W[f, out]^T-as-lhsT  @  xT[f, tok]
tiled matmul with PSUM accumulation over the 32 contraction slices.
"""

import numpy as np
import ml_dtypes
from contextlib import ExitStack

import concourse.bass as bass
import concourse.bacc as bacc
import concourse.tile as tile
from concourse import mybir
from concourse.bass_utils import run_bass_kernel_spmd

# Problem constants (hardcoded per spec)
N_TOK = 8192
R = 256  # out block-rows
C = 256  # in block-cols
K = 64   # kept blocks per row
B = 16   # block size
D_IN = C * B   # 4096
D_OUT = R * B  # 4096
NCORES = 8
TOK = N_TOK // NCORES  # 1024 tokens per core
TT = 2                 # token tiles per core (512 each)
TOKT = TOK // TT       # 512

# matmul dtype: "bf16" or "f32r"
MM_DTYPE = "bf16"

_cache = {}


def _build_program(feat_tiles: int, dt_name: str):
    """Build the SPMD single-core program. Returns (nc, names)."""
    key = (feat_tiles, dt_name)
    if key in _cache:
        return _cache[key]

    if dt_name == "bf16":
        mmdt = mybir.dt.bfloat16
    else:
        mmdt = mybir.dt.float32r
    f32 = mybir.dt.float32

    FEAT = feat_tiles          # contraction tiles of 128
    OGB = 16                   # out-group blocks (256 outs each)
    OG2 = 2                    # 128-out groups per block

    nc = bacc.Bacc("TRN2", target_bir_lowering=False, debug=False,
                   num_devices=NCORES)

    xT_d = nc.dram_tensor("xT", [FEAT * 128, TOK], mmdt, kind="ExternalInput").ap()
    # W pre-tiled on host: [OGB, FEAT, 128, 256] so each (ogb, f) tile is contiguous
    w_d = nc.dram_tensor("W", [OGB, FEAT, 128, 256], mmdt, kind="ExternalInput").ap()
    yT_d = nc.dram_tensor("yT", [D_OUT, TOK], f32, kind="ExternalOutput").ap()

    with tile.TileContext(nc) as tc, ExitStack() as ctx:
        xpool = ctx.enter_context(tc.tile_pool(name="x", bufs=1))
        wpool = ctx.enter_context(tc.tile_pool(name="w", bufs=12))
        ppool = ctx.enter_context(tc.tile_pool(name="ps", bufs=2, space="PSUM"))
        ypool = ctx.enter_context(tc.tile_pool(name="y", bufs=3))

        # resident x^T: [128, FEAT*TOK], slice f at cols f*TOK..(f+1)*TOK
        xt = xpool.tile([128, FEAT * TOK], mmdt)
        xT_r = xT_d.rearrange("(f p) n -> f p n", p=128)
        for f in range(FEAT):
            nc.sync.dma_start(out=xt[:, f * TOK:(f + 1) * TOK], in_=xT_r[f])

        # PE warmup while x streams in: absorbs the cold HAM clock-gate
        # phase (~3.4us at half rate) with throwaway work so the first real
        # matmuls run at full speed. Longer warmups serialize ahead of real
        # work (PE FIFO) and regress.
        NWARM = 72
        wj = xpool.tile([128, 128], mmdt, name="wj")
        nc.vector.memset(wj[:], 0.0)
        wu = ppool.tile([128, 128], f32, name="wu", tag="ps0")
        for i in range(NWARM):
            nc.tensor.matmul(wu[:], wj[:], wj[:, :128],
                             start=(i == 0), stop=(i == NWARM - 1))

        for ogb in range(OGB):
            ps = [ppool.tile([128, TOKT], f32, name=f"ps_{ogb}_{i}", tag=f"ps{i}")
                  for i in range(4)]
            for f in range(FEAT):
                # W streams on the scalar HWDGE ring so it is not queued
                # behind the resident-x loads on the sync ring
                wt = wpool.tile([128, 256], mmdt, name=f"wt_{ogb}_{f}", tag="wt")
                nc.scalar.dma_start(out=wt[:], in_=w_d[ogb, f])
                for og2 in range(OG2):
                    for t in range(TT):
                        nc.tensor.matmul(
                            ps[og2 * TT + t][:],
                            wt[:, og2 * 128:(og2 + 1) * 128],
                            xt[:, f * TOK + t * TOKT: f * TOK + (t + 1) * TOKT],
                            start=(f == 0), stop=(f == FEAT - 1),
                        )
            for og2 in range(OG2):
                yt = ypool.tile([128, TOK], f32, name=f"yt_{ogb}_{og2}", tag="yt")
                og = ogb * OG2 + og2
                for t in range(TT):
                    if (og2 * TT + t) % 2 == 0:
                        nc.vector.tensor_copy(yt[:, t * TOKT:(t + 1) * TOKT],
                                              ps[og2 * TT + t][:])
                    else:
                        nc.scalar.copy(yt[:, t * TOKT:(t + 1) * TOKT],
                                       ps[og2 * TT + t][:])
                    # per-half store so the final DMA starts as soon as its
                    # eviction lands rather than after both halves
                    nc.sync.dma_start(
                        out=yT_d[og * 128:(og + 1) * 128,
                                 t * TOKT:(t + 1) * TOKT],
                        in_=yt[:, t * TOKT:(t + 1) * TOKT])

    nc.compile()
    _cache[key] = nc
    return nc


def _scatter_dense(values: np.ndarray, col_indices: np.ndarray) -> np.ndarray:
    """W[c*16+i, r*16+o] = sum_{k: col[r,k]=c} values[r,k,o,i]."""
    Wd = np.zeros((C, B, R, B), np.float32)  # [c, i, r, o]
    vT = np.ascontiguousarray(values.transpose(0, 1, 3, 2))  # [r, k, i, o]
    for r in range(R):
        np.add.at(Wd[:, :, r, :], (col_indices[r],), vT[r])
    return Wd.reshape(D_IN, D_OUT)


def _run(x, values, bias, col_indices, trace=False):
    x = np.asarray(x, np.float32)
    values = np.asarray(values, np.float32)
    bias = np.asarray(bias, np.float32)
    col_indices = np.asarray(col_indices, np.int32)

    W = _scatter_dense(values, col_indices)  # [D_IN, D_OUT] fp32
    has_bias = bool(np.any(bias))
    FEAT = D_IN // 128 + (1 if has_bias else 0)

    # augment contraction with a bias row if needed
    xT = np.ascontiguousarray(x.T)  # [D_IN, N_TOK]
    if has_bias:
        xT = np.concatenate([xT, np.zeros((128, N_TOK), np.float32)], 0)
        xT[D_IN, :] = 1.0
        W = np.concatenate([W, np.zeros((128, D_OUT), np.float32)], 0)
        W[D_IN, :] = bias

    np_dt = ml_dtypes.bfloat16 if MM_DTYPE == "bf16" else np.float32
    # pre-tile W: [OGB, FEAT, 128, 256]
    Wt = np.ascontiguousarray(
        W.reshape(FEAT, 128, 16, 256).transpose(2, 0, 1, 3)).astype(np_dt)
    xTc = xT.astype(np_dt)

    nc = _build_program(FEAT, MM_DTYPE)

    in_maps = []
    for c in range(NCORES):
        shard = np.ascontiguousarray(xTc[:, c * TOK:(c + 1) * TOK])
        in_maps.append({"xT": shard, "W": Wt})

    res = run_bass_kernel_spmd(nc, in_maps, list(range(NCORES)), trace=trace)

    y = np.empty((N_TOK, D_OUT), np.float32)
    for c in range(NCORES):
        y[c * TOK:(c + 1) * TOK, :] = res.results[c]["yT"].T
    return y, res


def kernel(x: np.ndarray, values: np.ndarray, bias: np.ndarray,
           col_indices: np.ndarray) -> np.ndarray:
    return _run(x, values, bias, col_indices)[0]


def run_traced(x, values, bias, col_indices):
    return _run(x, values, bias, col_indices, trace=True)[1]
